# revision 2
# baseline (speedup 1.0000x reference)
"""GAT-style masked self-attention (B=4, N=4096, D=128) on 8 trn2 NeuronCores.

reference:
    scores = X @ X^T / sqrt(D)            [B, N, N]
    masked = where(adj > 0, scores, -1e12)
    attn   = softmax(masked, axis=2)
    out    = attn @ X                     [B, N, D]

Sharding: 8 cores <- (batch b, row-half h); each core handles 2048 rows
of one batch element against all 4096 keys. No collectives.

Key order is rolled per core so the core's own rows are keys [0:R) --
softmax is key-order invariant, and this lets one SPMD program slice its
row block out of the same xt buffer on every core (no separate xtr DMA).

Device algorithm (per core), keys on partitions:
  - score matmul (fp16, full PE rate): psS = xt[:,k128].T @ xt[:,rows]
  - eviction+softmax work is split across TWO engines to balance the
    elementwise bottleneck (the old all-ACT eviction was the critical
    path at ~72us busy):
      * ACT supergroups: ACT evicts PSUM with exp fused (fp16), then DVE
        applies a multiplicative 0/1 fp8e4m3 mask at 2x rate.
      * DVE supergroups: ONE fused scalar_tensor_tensor per psS tile:
        u16 = uint16_sat(psS * A + madd), madd in {3584, -28672} fp8e5m2.
        The uint16 result IS the fp16 bit pattern of 2^(t/1024-15) ~
        exp(score*SCALE)*2^-11.5 (Schraudolph bit trick; the convert
        saturates negatives to 0, which both applies the mask and
        implements prob underflow). Sawtooth mean factor gbar is folded
        into the ACT path's exp bias so the two paths share one global
        scale that cancels in the softmax ratio.
  - AV matmul with the denominator fused via an appended ones-column:
      psO[rc] (+)= ptm[:, k, rc128].T @ [X_k | 1]   accumulated over k
      out = psO[:, :128] * (1 / psO[:, 128])
  - row blocks are software-pipelined: block i runs scores/evict while
    block i-1 runs its AV matmuls; AV matmuls are emitted first within
    each group so PE covers the eviction drain. The last two blocks are
    256 rows so the final (unoverlapped) AV drain is short.
"""

import math
import sys

sys.path.insert(0, "/opt/trn_rl_repo")

import numpy as np

B, N, D = 4, 4096, 128
R = N // 2            # rows per core
NK = N // 128         # 32 key tiles
SG = 8                # key tiles per super group (one mask DMA / mask op)
NSG = NK // SG
SCALE = 1.0 / math.sqrt(D)

# fast-exp bit trick constants (DVE supergroups)
A_TRICK = 1024.0 * math.log2(math.e) * SCALE      # 130.5778...
MADD_UNMASK = 3584.0                              # exact in fp8e5m2
MADD_MASK = -28672.0                              # exact in fp8e5m2
GBAR = 0.5 / math.log(2.0) ** 2                   # sawtooth mean 1.04068
# ACT path: exp(s*SCALE + EXP_BIAS) == GBAR * 2^(3584/1024 - 15) * exp(s*SCALE)
EXP_BIAS = -11.5 * math.log(2.0) + math.log(GBAR)  # -7.93131

# row blocks (offset, size): last two halved to shorten the AV drain tail
BLOCKS = [(0, 512), (512, 512), (1024, 512), (1536, 256), (1792, 256)]

CFG = dict(
    p_dt="float16",
    act_mask_dt="float8e4",   # multiplicative 0/1 mask for ACT supergroups
    dve_mask_dt="float8e5",   # additive mask for DVE supergroups
    kg=2,                     # key tiles per PSUM score tile
    ptm_bufs=2,
    psum_s_bufs=2,
    adj_bufs=4,
    # (phase, sg) pairs evicted by the fused DVE trick; the rest go
    # through ACT exp. Chosen off the diagonal supergroups (blk0/1: sg0,
    # blk2/3/4: sg1) so dominant diagonal probs stay exact.
    dve_sgs=((0, 2), (1, 3), (2, 2), (3, 3), (4, 2)),
)

_CACHE = {}


def _mask_chunks(cfg):
    """Walk (phase, sg) in program order; assign flat column offsets in the
    per-kind mask tensors. Returns per-(phase,sg) records and totals."""
    dve = set(cfg["dve_sgs"])
    recs = {}
    tot = {"act": 0, "dve": 0}
    for phase, (off, bs) in enumerate(BLOCKS):
        for sg in range(NSG):
            kind = "dve" if (phase, sg) in dve else "act"
            recs[(phase, sg)] = (kind, tot[kind], SG * bs)
            tot[kind] += SG * bs
    return recs, tot


def _build_nc(cfg):
    from concourse import bacc
    import concourse.mybir as mybir
    from concourse.tile import TileContext

    dt = mybir.dt
    p_dt = getattr(dt, cfg["p_dt"])
    am_dt = getattr(dt, cfg["act_mask_dt"])
    dm_dt = getattr(dt, cfg["dve_mask_dt"])
    kg = cfg["kg"]
    recs, tot = _mask_chunks(cfg)

    nc = bacc.Bacc(None, target_bir_lowering=False)

    xt_d = nc.dram_tensor("xt", [D, N], p_dt, kind="ExternalInput")
    xaug_d = nc.dram_tensor("xaug", [N, D + 1], p_dt, kind="ExternalInput")
    adjm_d = nc.dram_tensor("adjm", [128, max(tot["act"], 1)], am_dt,
                            kind="ExternalInput")
    adja_d = nc.dram_tensor("adja", [128, max(tot["dve"], 1)], dm_dt,
                            kind="ExternalInput")
    o_d = nc.dram_tensor("o", [R, D], dt.float32, kind="ExternalOutput")

    with TileContext(nc) as tc:
        with (
            tc.tile_pool(name="singles", bufs=1) as singles,
            tc.tile_pool(name="ptm", bufs=cfg["ptm_bufs"]) as ptm_pool,
            tc.tile_pool(name="adj", bufs=cfg["adj_bufs"]) as adj_pool,
            tc.tile_pool(name="pe", bufs=3) as pe_pool,
            tc.tile_pool(name="outs", bufs=4) as out_pool,
            tc.tile_pool(name="small", bufs=4) as small_pool,
            tc.tile_pool(name="psS", bufs=cfg["psum_s_bufs"], space="PSUM") as psS_pool,
            tc.tile_pool(name="psO", bufs=4, space="PSUM") as psO_pool,
        ):
            ebias = singles.tile([128, 1], mybir.dt.float32)
            nc.vector.memset(ebias[:], EXP_BIAS)
            # warm the exp table while the init DMAs stream in
            warm = small_pool.tile([128, 1], mybir.dt.float32, tag="warm")
            nc.vector.memset(warm[:], 0.0)
            warm2 = small_pool.tile([128, 1], mybir.dt.float32, tag="warm")
            nc.scalar.activation(
                warm2[:], warm[:], mybir.ActivationFunctionType.Exp, scale=1.0
            )

            xt_sb = singles.tile([D, N], p_dt)
            xaug_sb = singles.tile([128, NK, D + 1], p_dt)

            def adj_dma(phase, sg, bs, tile):
                kind, c0, ncols = recs[(phase, sg)]
                src = adjm_d if kind == "act" else adja_d
                nc.sync.dma_start(out=tile[:], in_=src[:, c0:c0 + ncols])

            def adj_tile(phase, sg, bs):
                kind = recs[(phase, sg)][0]
                adt = am_dt if kind == "act" else dm_dt
                return adj_pool.tile([128, SG, bs], adt, tag="adj",
                                     name=f"adj_{phase}_{sg}")

            # init DMAs staggered by first consumption: the first score
            # matmuls need xt[0:1024] (covers block-0 rows too, thanks to
            # the rolled key order) and block 0's first mask chunk.
            nc.sync.dma_start(out=xt_sb[:, 0:1024], in_=xt_d[:, 0:1024])
            adj_sbs0 = []
            a0 = adj_tile(0, 0, 512)
            adj_dma(0, 0, 512, a0)
            adj_sbs0.append(a0)
            nc.sync.dma_start(out=xt_sb[:, 1024:2048], in_=xt_d[:, 1024:2048])
            a1 = adj_tile(0, 1, 512)
            adj_dma(0, 1, 512, a1)
            nc.sync.dma_start(out=xt_sb[:, 2048:3072], in_=xt_d[:, 2048:3072])
            adj_sbs0.append(a1)
            a2 = adj_tile(0, 2, 512)
            adj_dma(0, 2, 512, a2)
            nc.sync.dma_start(out=xt_sb[:, 3072:4096], in_=xt_d[:, 3072:4096])
            a3 = adj_tile(0, 3, 512)
            adj_dma(0, 3, 512, a3)
            adj_sbs0 += [a2, a3]
            nc.gpsimd.dma_start(
                out=xaug_sb[:],
                in_=xaug_d[:, :].rearrange("(t p) d -> p t d", p=128),
            )

            dve = set(cfg["dve_sgs"])
            NB = len(BLOCKS)
            ptm_prev = None
            bs_prev = None
            off_prev = None
            for phase in range(NB + 1):
                ptm_cur = None
                psO = None
                adj_sbs = []
                if phase < NB:
                    off, bs = BLOCKS[phase]
                    ptm_cur = ptm_pool.tile([128, NK, bs], p_dt, tag="ptm",
                                            name=f"ptm_{phase}")
                    if phase == 0:
                        adj_sbs = adj_sbs0
                    else:
                        for sg in range(NSG):
                            a = adj_tile(phase, sg, bs)
                            adj_dma(phase, sg, bs, a)
                            adj_sbs.append(a)
                if phase >= 1:
                    psO = [
                        psO_pool.tile(
                            [128, D + 1], mybir.dt.float32,
                            tag="psO", name=f"psO_{phase}_{rc}",
                        )
                        for rc in range(bs_prev // 128)
                    ]

                if phase == NB:
                    # drain: rc-major AV bursts so each psO finishes early
                    # and its normalize/store overlaps the next burst
                    for rc in range(bs_prev // 128):
                        for k in range(NK):
                            nc.tensor.matmul(
                                psO[rc][:, :],
                                lhsT=ptm_prev[:, k, rc * 128:(rc + 1) * 128],
                                rhs=xaug_sb[:, k, :],
                                start=(k == 0),
                                stop=(k == NK - 1),
                            )
                        recip = small_pool.tile([128, 1], mybir.dt.float32,
                                                tag="recip", name=f"recipd_{rc}")
                        nc.vector.reciprocal(recip[:], psO[rc][:, D:D + 1])
                        o_sb = out_pool.tile([128, D], mybir.dt.float32, tag="o",
                                             name=f"od_{rc}")
                        nc.vector.tensor_scalar_mul(o_sb[:], psO[rc][:, 0:D],
                                                    recip[:])
                        r0 = off_prev + rc * 128
                        nc.sync.dma_start(out=o_d[r0:r0 + 128, :], in_=o_sb[:])
                    break

                kg_b = kg * (512 // bs)   # keep kg_b*bs = 1024 elems per evict
                for sg in range(NSG):
                    is_dve = (phase, sg) in dve
                    k0 = sg * SG
                    pet = None
                    if not is_dve:
                        pet = pe_pool.tile([128, SG, bs], p_dt, tag="pe",
                                           name=f"pe_{phase}_{sg}")
                    for kgi in range(SG // kg_b):
                        # AV matmuls for the previous block first: PE has
                        # work while the evictions drain this group.
                        if phase >= 1:
                            for j in range(kg_b):
                                k = sg * SG + kgi * kg_b + j
                                for rc in range(bs_prev // 128):
                                    nc.tensor.matmul(
                                        psO[rc][:, :],
                                        lhsT=ptm_prev[:, k, rc * 128:(rc + 1) * 128],
                                        rhs=xaug_sb[:, k, :],
                                        start=(k == 0),
                                        stop=(k == NK - 1),
                                    )
                        ps = psS_pool.tile([128, kg_b, bs], mybir.dt.float32,
                                           tag="psS", name=f"psS_{phase}_{sg}_{kgi}")
                        for j in range(kg_b):
                            k = sg * SG + kgi * kg_b + j
                            nc.tensor.matmul(
                                ps[:, j, :],
                                lhsT=xt_sb[:, k * 128:(k + 1) * 128],
                                rhs=xt_sb[:, off:off + bs],
                                start=True,
                                stop=True,
                            )
                        if is_dve:
                            # fused evict+exp+mask: uint16 bit-trick, the
                            # saturating convert zeroes masked/underflowed
                            nc.vector.scalar_tensor_tensor(
                                ptm_cur[:, k0 + kgi * kg_b:
                                        k0 + (kgi + 1) * kg_b, :].bitcast(
                                            mybir.dt.uint16),
                                ps[:, :, :],
                                A_TRICK,
                                adj_sbs[sg][:, kgi * kg_b:(kgi + 1) * kg_b, :],
                                op0=mybir.AluOpType.mult,
                                op1=mybir.AluOpType.add,
                            )
                        else:
                            nc.scalar.activation(
                                pet[:, kgi * kg_b:(kgi + 1) * kg_b, :],
                                ps[:, :, :],
                                mybir.ActivationFunctionType.Exp,
                                bias=ebias[:],
                                scale=SCALE,
                            )
                    if not is_dve:
                        nc.vector.tensor_mul(
                            ptm_cur[:, k0:k0 + SG, :],
                            pet[:, :, :],
                            adj_sbs[sg][:, :, :],
                        )
                if phase >= 1:
                    for rc in range(bs_prev // 128):
                        recip = small_pool.tile([128, 1], mybir.dt.float32,
                                                tag="recip",
                                                name=f"recip_{phase}_{rc}")
                        nc.vector.reciprocal(recip[:], psO[rc][:, D:D + 1])
                        o_sb = out_pool.tile([128, D], mybir.dt.float32, tag="o",
                                             name=f"o_{phase}_{rc}")
                        nc.vector.tensor_scalar_mul(o_sb[:], psO[rc][:, 0:D],
                                                    recip[:])
                        r0 = off_prev + rc * 128
                        nc.sync.dma_start(out=o_d[r0:r0 + 128, :], in_=o_sb[:])
                ptm_prev = ptm_cur
                bs_prev = bs
                off_prev = off
    nc.finalize()
    return nc


def _get_nc():
    key = str(sorted(CFG.items()))
    if key not in _CACHE:
        _CACHE[key] = _build_nc(CFG)
    return _CACHE[key]


def _np_dt(name):
    import ml_dtypes

    return {
        "float32": np.float32,
        "bfloat16": ml_dtypes.bfloat16,
        "float16": np.float16,
        "float8e4": ml_dtypes.float8_e4m3,
        "float8e5": ml_dtypes.float8_e5m2,
    }[name]


def make_in_maps(input, adj):
    """Host-side shard/layout prep: one input map per core."""
    input = np.asarray(input, dtype=np.float32)
    adj = np.asarray(adj)
    p_np = _np_dt(CFG["p_dt"])
    am_np = _np_dt(CFG["act_mask_dt"])
    dm_np = _np_dt(CFG["dve_mask_dt"])
    recs, tot = _mask_chunks(CFG)

    in_maps = []
    for core in range(8):
        b, h = core // 2, core % 2
        xb = input[b]                                    # [N, D]
        roll = np.concatenate([np.arange(h * R, N), np.arange(0, h * R)])
        xr = xb[roll]                                    # keys rolled
        xt = np.ascontiguousarray(xr.T).astype(p_np, copy=False)
        xaug = np.concatenate([xr, np.ones((N, 1), np.float32)], axis=1)
        xaug = np.ascontiguousarray(xaug).astype(p_np)
        # mask[r, j] = adj[b][h*R + r, roll[j]] > 0; rows r are global
        mrows = adj[b][h * R:(h + 1) * R][:, roll] > 0   # [R, N] bool
        adjm = np.empty((128, max(tot["act"], 1)), am_np)
        adja = np.empty((128, max(tot["dve"], 1)), dm_np)
        for phase, (off, bs) in enumerate(BLOCKS):
            for sg in range(NSG):
                kind, c0, ncols = recs[(phase, sg)]
                sub = mrows[off:off + bs, sg * SG * 128:(sg + 1) * SG * 128]
                # [bs, SG, 128] -> [128(key%128), SG, bs] -> flat (SG, bs)
                subt = sub.reshape(bs, SG, 128).transpose(2, 1, 0)
                flat = subt.reshape(128, ncols)
                if kind == "act":
                    adjm[:, c0:c0 + ncols] = flat.astype(am_np)
                else:
                    adja[:, c0:c0 + ncols] = np.where(
                        flat, MADD_UNMASK, MADD_MASK
                    ).astype(dm_np)
        in_maps.append({"xt": xt, "xaug": xaug, "adjm": adjm, "adja": adja})
    return in_maps


def run_device(in_maps, trace=False, trace_cores=None):
    import concourse.bass_utils as bass_utils

    if trace:
        bass_utils.upload_artifacts = lambda tmpdir: ""  # no bucket in sandbox
    nc = _get_nc()
    return bass_utils.run_bass_kernel_spmd(
        nc, in_maps, list(range(8)), trace=trace, trace_cores=trace_cores
    )


def kernel(input, adj):
    res = run_device(make_in_maps(input, adj))
    out = np.empty((B, N, D), dtype=np.float32)
    for core in range(8):
        b, h = core // 2, core % 2
        out[b, h * R:(h + 1) * R, :] = res.results[core]["o"]
    return out


# revision 3
# speedup vs baseline: 1.1209x; 1.1209x over previous
"""GAT-style masked self-attention (B=4, N=4096, D=128) on 8 trn2 NeuronCores.

reference:
    scores = X @ X^T / sqrt(D)            [B, N, N]
    masked = where(adj > 0, scores, -1e12)
    attn   = softmax(masked, axis=2)
    out    = attn @ X                     [B, N, D]

Sharding: 8 cores <- (batch b, row-half h); each core handles 2048 rows
of one batch element against all 4096 keys. No collectives.

Key order is rolled per core so the core's own rows are keys [0:R) --
softmax is key-order invariant, and this lets one SPMD program slice its
row block out of the same xt buffer on every core (no separate xtr DMA).

Device algorithm (per core), keys on partitions:
  - score matmul (fp16, full PE rate): psS = xt[:,k128].T @ xt[:,rows]
  - eviction+softmax work is split across TWO engines to balance the
    elementwise bottleneck (the old all-ACT eviction was the critical
    path at ~72us busy):
      * ACT supergroups: ACT evicts PSUM with exp fused (fp16), then DVE
        applies a multiplicative 0/1 fp8e4m3 mask at 2x rate.
      * DVE supergroups: ONE fused scalar_tensor_tensor per psS tile:
        u16 = uint16_sat(psS * A + madd), madd in {3584, -28672} fp8e5m2.
        The uint16 result IS the fp16 bit pattern of 2^(t/1024-15) ~
        exp(score*SCALE)*2^-11.5 (Schraudolph bit trick; the convert
        saturates negatives to 0, which both applies the mask and
        implements prob underflow). Sawtooth mean factor gbar is folded
        into the ACT path's exp bias so the two paths share one global
        scale that cancels in the softmax ratio.
  - AV matmul with the denominator fused via an appended ones-column:
      psO[rc] (+)= ptm[:, k, rc128].T @ [X_k | 1]   accumulated over k
      out = psO[:, :128] * (1 / psO[:, 128])
  - row blocks are software-pipelined: block i runs scores/evict while
    block i-1 runs its AV matmuls; AV matmuls are emitted first within
    each group so PE covers the eviction drain. The last two blocks are
    256 rows so the final (unoverlapped) AV drain is short.
"""

import math
import sys

sys.path.insert(0, "/opt/trn_rl_repo")

import numpy as np

B, N, D = 4, 4096, 128
R = N // 2            # rows per core
NK = N // 128         # 32 key tiles
SG = 8                # key tiles per super group (one mask DMA / mask op)
NSG = NK // SG
SCALE = 1.0 / math.sqrt(D)

# fast-exp bit trick constants (DVE supergroups)
A_TRICK = 1024.0 * math.log2(math.e) * SCALE      # 130.5778...
MADD_UNMASK = 3584.0                              # exact in fp8e5m2
MADD_MASK = -28672.0                              # exact in fp8e5m2
GBAR = 0.5 / math.log(2.0) ** 2                   # sawtooth mean 1.04068
# ACT path: exp(s*SCALE + EXP_BIAS) == GBAR * 2^(3584/1024 - 15) * exp(s*SCALE)
EXP_BIAS = -11.5 * math.log(2.0) + math.log(GBAR)  # -7.93131

# row blocks (offset, size): last two halved to shorten the AV drain tail
BLOCKS = [(0, 512), (512, 512), (1024, 512), (1536, 256), (1792, 256)]

CFG = dict(
    p_dt="float16",
    act_mask_dt="float16",    # multiplicative 0/1 mask for ACT supergroups
                              # (must be 2-byte: tensor_tensor 2x needs it)
    dve_mask_dt="float8e5",   # additive mask for DVE supergroups
    kg=2,                     # key tiles per PSUM score tile
    ptm_bufs=2,
    psum_s_bufs=2,
    adj_bufs=4,
    # (phase, sg) pairs evicted by the fused DVE trick; the rest go
    # through ACT exp. Chosen off the diagonal supergroups (blk0/1: sg0,
    # blk2/3/4: sg1) so dominant diagonal probs stay exact.
    dve_sgs=((0, 2), (1, 3), (2, 2), (3, 3), (4, 2)),
)

_CACHE = {}


def _mask_chunks(cfg):
    """Walk (phase, sg) in program order; assign flat column offsets in the
    per-kind mask tensors. Returns per-(phase,sg) records and totals."""
    dve = set(cfg["dve_sgs"])
    recs = {}
    tot = {"act": 0, "dve": 0}
    for phase, (off, bs) in enumerate(BLOCKS):
        for sg in range(NSG):
            kind = "dve" if (phase, sg) in dve else "act"
            recs[(phase, sg)] = (kind, tot[kind], SG * bs)
            tot[kind] += SG * bs
    return recs, tot


def _build_nc(cfg):
    from concourse import bacc
    import concourse.mybir as mybir
    from concourse.tile import TileContext

    dt = mybir.dt
    p_dt = getattr(dt, cfg["p_dt"])
    am_dt = getattr(dt, cfg["act_mask_dt"])
    dm_dt = getattr(dt, cfg["dve_mask_dt"])
    kg = cfg["kg"]
    recs, tot = _mask_chunks(cfg)

    nc = bacc.Bacc(None, target_bir_lowering=False)

    xt_d = nc.dram_tensor("xt", [D, N], p_dt, kind="ExternalInput")
    xaug_d = nc.dram_tensor("xaug", [N, D + 1], p_dt, kind="ExternalInput")
    adjm_d = nc.dram_tensor("adjm", [128, max(tot["act"], 1)], am_dt,
                            kind="ExternalInput")
    adja_d = nc.dram_tensor("adja", [128, max(tot["dve"], 1)], dm_dt,
                            kind="ExternalInput")
    o_d = nc.dram_tensor("o", [R, D], dt.float32, kind="ExternalOutput")

    with TileContext(nc) as tc:
        with (
            tc.tile_pool(name="singles", bufs=1) as singles,
            tc.tile_pool(name="ptm", bufs=cfg["ptm_bufs"]) as ptm_pool,
            tc.tile_pool(name="adj", bufs=cfg["adj_bufs"]) as adj_pool,
            tc.tile_pool(name="pe", bufs=3) as pe_pool,
            tc.tile_pool(name="outs", bufs=4) as out_pool,
            tc.tile_pool(name="small", bufs=4) as small_pool,
            tc.tile_pool(name="psS", bufs=cfg["psum_s_bufs"], space="PSUM") as psS_pool,
            tc.tile_pool(name="psO", bufs=4, space="PSUM") as psO_pool,
        ):
            ebias = singles.tile([128, 1], mybir.dt.float32)
            nc.vector.memset(ebias[:], EXP_BIAS)
            # warm the exp table while the init DMAs stream in
            warm = small_pool.tile([128, 1], mybir.dt.float32, tag="warm")
            nc.vector.memset(warm[:], 0.0)
            warm2 = small_pool.tile([128, 1], mybir.dt.float32, tag="warm")
            nc.scalar.activation(
                warm2[:], warm[:], mybir.ActivationFunctionType.Exp, scale=1.0
            )

            xt_sb = singles.tile([D, N], p_dt)
            xaug_sb = singles.tile([128, NK, D + 1], p_dt)

            def adj_dma(phase, sg, bs, tile):
                kind, c0, ncols = recs[(phase, sg)]
                src = adjm_d if kind == "act" else adja_d
                nc.sync.dma_start(out=tile[:], in_=src[:, c0:c0 + ncols])

            def adj_tile(phase, sg, bs):
                kind = recs[(phase, sg)][0]
                adt = am_dt if kind == "act" else dm_dt
                return adj_pool.tile([128, SG, bs], adt, tag="adj",
                                     name=f"adj_{phase}_{sg}")

            # init DMAs staggered by first consumption: the first score
            # matmuls need xt[0:1024] (covers block-0 rows too, thanks to
            # the rolled key order) and block 0's first mask chunk.
            nc.sync.dma_start(out=xt_sb[:, 0:1024], in_=xt_d[:, 0:1024])
            adj_sbs0 = []
            a0 = adj_tile(0, 0, 512)
            adj_dma(0, 0, 512, a0)
            adj_sbs0.append(a0)
            nc.sync.dma_start(out=xt_sb[:, 1024:2048], in_=xt_d[:, 1024:2048])
            a1 = adj_tile(0, 1, 512)
            adj_dma(0, 1, 512, a1)
            nc.sync.dma_start(out=xt_sb[:, 2048:3072], in_=xt_d[:, 2048:3072])
            adj_sbs0.append(a1)
            a2 = adj_tile(0, 2, 512)
            adj_dma(0, 2, 512, a2)
            nc.sync.dma_start(out=xt_sb[:, 3072:4096], in_=xt_d[:, 3072:4096])
            a3 = adj_tile(0, 3, 512)
            adj_dma(0, 3, 512, a3)
            adj_sbs0 += [a2, a3]
            nc.gpsimd.dma_start(
                out=xaug_sb[:],
                in_=xaug_d[:, :].rearrange("(t p) d -> p t d", p=128),
            )

            dve = set(cfg["dve_sgs"])
            NB = len(BLOCKS)
            ptm_prev = None
            bs_prev = None
            off_prev = None
            for phase in range(NB + 1):
                ptm_cur = None
                psO = None
                adj_sbs = []
                if phase < NB:
                    off, bs = BLOCKS[phase]
                    ptm_cur = ptm_pool.tile([128, NK, bs], p_dt, tag="ptm",
                                            name=f"ptm_{phase}")
                    if phase == 0:
                        adj_sbs = adj_sbs0
                    else:
                        for sg in range(NSG):
                            a = adj_tile(phase, sg, bs)
                            adj_dma(phase, sg, bs, a)
                            adj_sbs.append(a)
                if phase >= 1:
                    psO = [
                        psO_pool.tile(
                            [128, D + 1], mybir.dt.float32,
                            tag="psO", name=f"psO_{phase}_{rc}",
                        )
                        for rc in range(bs_prev // 128)
                    ]

                if phase == NB:
                    # drain: rc-major AV bursts so each psO finishes early
                    # and its normalize/store overlaps the next burst
                    for rc in range(bs_prev // 128):
                        for k in range(NK):
                            nc.tensor.matmul(
                                psO[rc][:, :],
                                lhsT=ptm_prev[:, k, rc * 128:(rc + 1) * 128],
                                rhs=xaug_sb[:, k, :],
                                start=(k == 0),
                                stop=(k == NK - 1),
                            )
                        recip = small_pool.tile([128, 1], mybir.dt.float32,
                                                tag="recip", name=f"recipd_{rc}")
                        nc.vector.reciprocal(recip[:], psO[rc][:, D:D + 1])
                        o_sb = out_pool.tile([128, D], mybir.dt.float32, tag="o",
                                             name=f"od_{rc}")
                        nc.vector.tensor_scalar_mul(o_sb[:], psO[rc][:, 0:D],
                                                    recip[:])
                        r0 = off_prev + rc * 128
                        nc.sync.dma_start(out=o_d[r0:r0 + 128, :], in_=o_sb[:])
                    break

                kg_b = kg * (512 // bs)   # keep kg_b*bs = 1024 elems per evict
                for sg in range(NSG):
                    is_dve = (phase, sg) in dve
                    k0 = sg * SG
                    pet = None
                    if not is_dve:
                        pet = pe_pool.tile([128, SG, bs], p_dt, tag="pe",
                                           name=f"pe_{phase}_{sg}")
                    for kgi in range(SG // kg_b):
                        # AV matmuls for the previous block first: PE has
                        # work while the evictions drain this group.
                        if phase >= 1:
                            for j in range(kg_b):
                                k = sg * SG + kgi * kg_b + j
                                for rc in range(bs_prev // 128):
                                    nc.tensor.matmul(
                                        psO[rc][:, :],
                                        lhsT=ptm_prev[:, k, rc * 128:(rc + 1) * 128],
                                        rhs=xaug_sb[:, k, :],
                                        start=(k == 0),
                                        stop=(k == NK - 1),
                                    )
                        ps = psS_pool.tile([128, kg_b, bs], mybir.dt.float32,
                                           tag="psS", name=f"psS_{phase}_{sg}_{kgi}")
                        for j in range(kg_b):
                            k = sg * SG + kgi * kg_b + j
                            nc.tensor.matmul(
                                ps[:, j, :],
                                lhsT=xt_sb[:, k * 128:(k + 1) * 128],
                                rhs=xt_sb[:, off:off + bs],
                                start=True,
                                stop=True,
                            )
                        if is_dve:
                            # fused evict+exp+mask: uint16 bit-trick, the
                            # saturating convert zeroes masked/underflowed
                            nc.vector.scalar_tensor_tensor(
                                ptm_cur[:, k0 + kgi * kg_b:
                                        k0 + (kgi + 1) * kg_b, :].bitcast(
                                            mybir.dt.uint16),
                                ps[:, :, :],
                                A_TRICK,
                                adj_sbs[sg][:, kgi * kg_b:(kgi + 1) * kg_b, :],
                                op0=mybir.AluOpType.mult,
                                op1=mybir.AluOpType.add,
                            )
                        else:
                            nc.scalar.activation(
                                pet[:, kgi * kg_b:(kgi + 1) * kg_b, :],
                                ps[:, :, :],
                                mybir.ActivationFunctionType.Exp,
                                bias=ebias[:],
                                scale=SCALE,
                            )
                    if not is_dve:
                        nc.vector.tensor_mul(
                            ptm_cur[:, k0:k0 + SG, :],
                            pet[:, :, :],
                            adj_sbs[sg][:, :, :],
                        )
                if phase >= 1:
                    for rc in range(bs_prev // 128):
                        recip = small_pool.tile([128, 1], mybir.dt.float32,
                                                tag="recip",
                                                name=f"recip_{phase}_{rc}")
                        nc.vector.reciprocal(recip[:], psO[rc][:, D:D + 1])
                        o_sb = out_pool.tile([128, D], mybir.dt.float32, tag="o",
                                             name=f"o_{phase}_{rc}")
                        nc.vector.tensor_scalar_mul(o_sb[:], psO[rc][:, 0:D],
                                                    recip[:])
                        r0 = off_prev + rc * 128
                        nc.sync.dma_start(out=o_d[r0:r0 + 128, :], in_=o_sb[:])
                ptm_prev = ptm_cur
                bs_prev = bs
                off_prev = off
    nc.finalize()
    return nc


def _get_nc():
    key = str(sorted(CFG.items()))
    if key not in _CACHE:
        _CACHE[key] = _build_nc(CFG)
    return _CACHE[key]


def _np_dt(name):
    import ml_dtypes

    return {
        "float32": np.float32,
        "bfloat16": ml_dtypes.bfloat16,
        "float16": np.float16,
        "float8e4": ml_dtypes.float8_e4m3,
        "float8e5": ml_dtypes.float8_e5m2,
    }[name]


def make_in_maps(input, adj):
    """Host-side shard/layout prep: one input map per core."""
    input = np.asarray(input, dtype=np.float32)
    adj = np.asarray(adj)
    p_np = _np_dt(CFG["p_dt"])
    am_np = _np_dt(CFG["act_mask_dt"])
    dm_np = _np_dt(CFG["dve_mask_dt"])
    recs, tot = _mask_chunks(CFG)

    in_maps = []
    for core in range(8):
        b, h = core // 2, core % 2
        xb = input[b]                                    # [N, D]
        roll = np.concatenate([np.arange(h * R, N), np.arange(0, h * R)])
        xr = xb[roll]                                    # keys rolled
        xt = np.ascontiguousarray(xr.T).astype(p_np, copy=False)
        xaug = np.concatenate([xr, np.ones((N, 1), np.float32)], axis=1)
        xaug = np.ascontiguousarray(xaug).astype(p_np)
        # mask[r, j] = adj[b][h*R + r, roll[j]] > 0; rows r are global
        mrows = adj[b][h * R:(h + 1) * R][:, roll] > 0   # [R, N] bool
        adjm = np.empty((128, max(tot["act"], 1)), am_np)
        adja = np.empty((128, max(tot["dve"], 1)), dm_np)
        for phase, (off, bs) in enumerate(BLOCKS):
            for sg in range(NSG):
                kind, c0, ncols = recs[(phase, sg)]
                sub = mrows[off:off + bs, sg * SG * 128:(sg + 1) * SG * 128]
                # [bs, SG, 128] -> [128(key%128), SG, bs] -> flat (SG, bs)
                subt = sub.reshape(bs, SG, 128).transpose(2, 1, 0)
                flat = subt.reshape(128, ncols)
                if kind == "act":
                    adjm[:, c0:c0 + ncols] = flat.astype(am_np)
                else:
                    adja[:, c0:c0 + ncols] = np.where(
                        flat, MADD_UNMASK, MADD_MASK
                    ).astype(dm_np)
        in_maps.append({"xt": xt, "xaug": xaug, "adjm": adjm, "adja": adja})
    return in_maps


def run_device(in_maps, trace=False, trace_cores=None):
    import concourse.bass_utils as bass_utils

    if trace:
        bass_utils.upload_artifacts = lambda tmpdir: ""  # no bucket in sandbox
    nc = _get_nc()
    return bass_utils.run_bass_kernel_spmd(
        nc, in_maps, list(range(8)), trace=trace, trace_cores=trace_cores
    )


def kernel(input, adj):
    res = run_device(make_in_maps(input, adj))
    out = np.empty((B, N, D), dtype=np.float32)
    for core in range(8):
        b, h = core // 2, core % 2
        out[b, h * R:(h + 1) * R, :] = res.results[core]["o"]
    return out


# revision 10
# speedup vs baseline: 1.1513x; 1.0271x over previous
"""GAT-style masked self-attention (B=4, N=4096, D=128) on 8 trn2 NeuronCores.

reference:
    scores = X @ X^T / sqrt(D)            [B, N, N]
    masked = where(adj > 0, scores, -1e12)
    attn   = softmax(masked, axis=2)
    out    = attn @ X                     [B, N, D]

Sharding: 8 cores <- (batch b, row-half h); each core handles 2048 rows
of one batch element against all 4096 keys. No collectives.

Key order is rolled per core so the core's own rows are keys [0:R) --
softmax is key-order invariant, and this lets one SPMD program slice its
row block out of the same xt buffer on every core (no separate xtr DMA).

Device algorithm (per core), keys on partitions:
  - score matmul (fp16, full PE rate): psS = xt[:,k128].T @ xt[:,rows]
  - eviction+softmax work is split across TWO engines to balance the
    elementwise bottleneck (the old all-ACT eviction was the critical
    path at ~72us busy):
      * ACT supergroups: ACT evicts PSUM with exp fused (fp16), then DVE
        applies a multiplicative 0/1 fp8e4m3 mask at 2x rate.
      * DVE supergroups: ONE fused scalar_tensor_tensor per psS tile:
        u16 = uint16_sat(psS * A + madd), madd in {3584, -28672} fp8e5m2.
        The uint16 result IS the fp16 bit pattern of 2^(t/1024-15) ~
        exp(score*SCALE)*2^-11.5 (Schraudolph bit trick; the convert
        saturates negatives to 0, which both applies the mask and
        implements prob underflow). Sawtooth mean factor gbar is folded
        into the ACT path's exp bias so the two paths share one global
        scale that cancels in the softmax ratio.
  - AV matmul with the denominator fused via an appended ones-column:
      psO[rc] (+)= ptm[:, k, rc128].T @ [X_k | 1]   accumulated over k
      out = psO[:, :128] * (1 / psO[:, 128])
  - row blocks are software-pipelined: block i runs scores/evict while
    block i-1 runs its AV matmuls; AV matmuls are emitted first within
    each group so PE covers the eviction drain. The last two blocks are
    256 rows so the final (unoverlapped) AV drain is short.
"""

import math
import sys

sys.path.insert(0, "/opt/trn_rl_repo")

import numpy as np

B, N, D = 4, 4096, 128
R = N // 2            # rows per core
NK = N // 128         # 32 key tiles
SG = 8                # key tiles per super group (one mask DMA / mask op)
NSG = NK // SG
SCALE = 1.0 / math.sqrt(D)

# fast-exp bit trick constants (DVE supergroups)
A_TRICK = 1024.0 * math.log2(math.e) * SCALE      # 130.5778...
MADD_UNMASK = 3584.0                              # exact in fp8e5m2
MADD_MASK = -28672.0                              # exact in fp8e5m2
GBAR = 0.5 / math.log(2.0) ** 2                   # sawtooth mean 1.04068
# ACT path: exp(s*SCALE + EXP_BIAS) == GBAR * 2^(3584/1024 - 15) * exp(s*SCALE)
EXP_BIAS = -11.5 * math.log(2.0) + math.log(GBAR)  # -7.93131

# row blocks (offset, size): first and last two halved to shorten the
# (DMA-bound) pipeline fill and the AV drain tail
BLOCKS = [(0, 256), (256, 256), (512, 512), (1024, 512),
          (1536, 256), (1792, 256)]

CFG = dict(
    p_dt="float16",
    act_mask_dt="float16",    # multiplicative 0/1 mask for ACT supergroups
                              # (must be 2-byte: tensor_tensor 2x needs it)
    dve_mask_dt="float8e5",   # additive mask for DVE supergroups
    kg=2,                     # key tiles per PSUM score tile
    ptm_bufs=2,
    psum_s_bufs=2,
    adj_bufs=6,
    # (phase, sg) pairs evicted by the fused DVE trick; the rest go
    # through ACT exp. Chosen off the diagonal supergroups so the
    # dominant diagonal probs stay exact.
    dve_sgs=((0, 1), (1, 2), (2, 3), (3, 2), (4, 3), (5, 2)),
)

_CACHE = {}


def _mask_chunks(cfg):
    """Walk (phase, sg) in program order; assign flat column offsets in the
    per-kind mask tensors. Returns per-(phase,sg) records and totals."""
    dve = set(cfg["dve_sgs"])
    recs = {}
    tot = {"act": 0, "dve": 0}
    for phase, (off, bs) in enumerate(BLOCKS):
        for sg in range(NSG):
            kind = "dve" if (phase, sg) in dve else "act"
            recs[(phase, sg)] = (kind, tot[kind], SG * bs)
            tot[kind] += SG * bs
    return recs, tot


def _build_nc(cfg):
    from concourse import bacc
    import concourse.mybir as mybir
    from concourse.tile import TileContext

    dt = mybir.dt
    p_dt = getattr(dt, cfg["p_dt"])
    am_dt = getattr(dt, cfg["act_mask_dt"])
    dm_dt = getattr(dt, cfg["dve_mask_dt"])
    kg = cfg["kg"]
    recs, tot = _mask_chunks(cfg)

    nc = bacc.Bacc(None, target_bir_lowering=False)

    xt_d = nc.dram_tensor("xt", [D, N], p_dt, kind="ExternalInput")
    # host pre-arranged to [128, NK, D+1] so the DMA is fully contiguous
    xaug_d = nc.dram_tensor("xaug", [128, NK, D + 1], p_dt,
                            kind="ExternalInput")
    adjm_d = nc.dram_tensor("adjm", [128, max(tot["act"], 1)], am_dt,
                            kind="ExternalInput")
    adja_d = nc.dram_tensor("adja", [128, max(tot["dve"], 1)], dm_dt,
                            kind="ExternalInput")
    o_d = nc.dram_tensor("o", [R, D], dt.float32, kind="ExternalOutput")

    with TileContext(nc) as tc:
        with (
            tc.tile_pool(name="singles", bufs=1) as singles,
            tc.tile_pool(name="ptm", bufs=cfg["ptm_bufs"]) as ptm_pool,
            tc.tile_pool(name="adj", bufs=cfg["adj_bufs"]) as adj_pool,
            tc.tile_pool(name="pe", bufs=3) as pe_pool,
            tc.tile_pool(name="outs", bufs=4) as out_pool,
            tc.tile_pool(name="small", bufs=4) as small_pool,
            tc.tile_pool(name="psS", bufs=cfg["psum_s_bufs"], space="PSUM") as psS_pool,
            tc.tile_pool(name="psO", bufs=4, space="PSUM") as psO_pool,
        ):
            ebias = singles.tile([128, 1], mybir.dt.float32)
            nc.vector.memset(ebias[:], EXP_BIAS)
            # warm the exp table while the init DMAs stream in
            warm = small_pool.tile([128, 1], mybir.dt.float32, tag="warm")
            nc.vector.memset(warm[:], 0.0)
            warm2 = small_pool.tile([128, 1], mybir.dt.float32, tag="warm")
            nc.scalar.activation(
                warm2[:], warm[:], mybir.ActivationFunctionType.Exp, scale=1.0
            )

            xt_sb = singles.tile([D, N], p_dt)
            xaug_sb = singles.tile([128, NK, D + 1], p_dt)

            def adj_dma(phase, sg, tile):
                kind, c0, ncols = recs[(phase, sg)]
                src = adjm_d if kind == "act" else adja_d
                # alternate DMA rings so the mask stream uses two queues
                eng = nc.sync if sg % 2 == 0 else nc.gpsimd
                eng.dma_start(out=tile[:], in_=src[:, c0:c0 + ncols])

            def adj_tile(phase, sg, bs):
                kind = recs[(phase, sg)][0]
                adt = am_dt if kind == "act" else dm_dt
                return adj_pool.tile([128, SG, bs], adt, tag="adj",
                                     name=f"adj_{phase}_{sg}")

            # init DMAs staggered by first consumption: the first score
            # matmuls need xt keys (block-0 rows are keys [0:256) thanks
            # to the rolled order) plus block 0's mask chunks. Masks
            # alternate between the sync and gpsimd rings.
            bs0 = BLOCKS[0][1]
            adj_sbs0 = [adj_tile(0, sg, bs0) for sg in range(NSG)]
            nc.sync.dma_start(out=xt_sb[:, 0:512], in_=xt_d[:, 0:512])
            adj_dma(0, 1, adj_sbs0[1])
            nc.sync.dma_start(out=xt_sb[:, 512:1024], in_=xt_d[:, 512:1024])
            adj_dma(0, 0, adj_sbs0[0])
            adj_dma(0, 3, adj_sbs0[3])
            nc.sync.dma_start(out=xt_sb[:, 1024:2048], in_=xt_d[:, 1024:2048])
            adj_dma(0, 2, adj_sbs0[2])
            nc.sync.dma_start(out=xt_sb[:, 2048:3072], in_=xt_d[:, 2048:3072])
            nc.gpsimd.dma_start(out=xaug_sb[:], in_=xaug_d[:, :, :])
            nc.sync.dma_start(out=xt_sb[:, 3072:4096], in_=xt_d[:, 3072:4096])

            dve = set(cfg["dve_sgs"])
            NB = len(BLOCKS)
            ptm_prev = None
            bs_prev = None
            off_prev = None
            for phase in range(NB + 1):
                ptm_cur = None
                psO = None
                adj_sbs = []
                if phase < NB:
                    off, bs = BLOCKS[phase]
                    ptm_cur = ptm_pool.tile([128, NK, bs], p_dt, tag="ptm",
                                            name=f"ptm_{phase}")
                    if phase == 0:
                        adj_sbs = adj_sbs0
                    else:
                        for sg in range(NSG):
                            a = adj_tile(phase, sg, bs)
                            adj_dma(phase, sg, a)
                            adj_sbs.append(a)
                if phase >= 1:
                    psO = [
                        psO_pool.tile(
                            [128, D + 1], mybir.dt.float32,
                            tag="psO", name=f"psO_{phase}_{rc}",
                        )
                        for rc in range(bs_prev // 128)
                    ]

                if phase == NB:
                    # drain: rc-major AV bursts so each psO finishes early
                    # and its normalize/store overlaps the next burst
                    for rc in range(bs_prev // 128):
                        for k in range(NK):
                            nc.tensor.matmul(
                                psO[rc][:, :],
                                lhsT=ptm_prev[:, k, rc * 128:(rc + 1) * 128],
                                rhs=xaug_sb[:, k, :],
                                start=(k == 0),
                                stop=(k == NK - 1),
                            )
                        recip = small_pool.tile([128, 1], mybir.dt.float32,
                                                tag="recip", name=f"recipd_{rc}")
                        nc.vector.reciprocal(recip[:], psO[rc][:, D:D + 1])
                        o_sb = out_pool.tile([128, D], mybir.dt.float32, tag="o",
                                             name=f"od_{rc}")
                        nc.vector.tensor_scalar_mul(o_sb[:], psO[rc][:, 0:D],
                                                    recip[:])
                        r0 = off_prev + rc * 128
                        nc.sync.dma_start(out=o_d[r0:r0 + 128, :], in_=o_sb[:])
                    break

                kg_b = kg * (512 // bs)   # keep kg_b*bs = 1024 elems per evict
                for sg in range(NSG):
                    is_dve = (phase, sg) in dve
                    k0 = sg * SG
                    pet = None
                    if not is_dve:
                        pet = pe_pool.tile([128, SG, bs], p_dt, tag="pe",
                                           name=f"pe_{phase}_{sg}")
                    for kgi in range(SG // kg_b):
                        # AV matmuls for the previous block first: PE has
                        # work while the evictions drain this group.
                        if phase >= 1:
                            for j in range(kg_b):
                                k = sg * SG + kgi * kg_b + j
                                for rc in range(bs_prev // 128):
                                    nc.tensor.matmul(
                                        psO[rc][:, :],
                                        lhsT=ptm_prev[:, k, rc * 128:(rc + 1) * 128],
                                        rhs=xaug_sb[:, k, :],
                                        start=(k == 0),
                                        stop=(k == NK - 1),
                                    )
                        ps = psS_pool.tile([128, kg_b, bs], mybir.dt.float32,
                                           tag="psS", name=f"psS_{phase}_{sg}_{kgi}")
                        for j in range(kg_b):
                            k = sg * SG + kgi * kg_b + j
                            nc.tensor.matmul(
                                ps[:, j, :],
                                lhsT=xt_sb[:, k * 128:(k + 1) * 128],
                                rhs=xt_sb[:, off:off + bs],
                                start=True,
                                stop=True,
                            )
                        if is_dve:
                            # fused evict+exp+mask: uint16 bit-trick, the
                            # saturating convert zeroes masked/underflowed
                            nc.vector.scalar_tensor_tensor(
                                ptm_cur[:, k0 + kgi * kg_b:
                                        k0 + (kgi + 1) * kg_b, :].bitcast(
                                            mybir.dt.uint16),
                                ps[:, :, :],
                                A_TRICK,
                                adj_sbs[sg][:, kgi * kg_b:(kgi + 1) * kg_b, :],
                                op0=mybir.AluOpType.mult,
                                op1=mybir.AluOpType.add,
                            )
                        else:
                            nc.scalar.activation(
                                pet[:, kgi * kg_b:(kgi + 1) * kg_b, :],
                                ps[:, :, :],
                                mybir.ActivationFunctionType.Exp,
                                bias=ebias[:],
                                scale=SCALE,
                            )
                    if not is_dve:
                        nc.vector.tensor_mul(
                            ptm_cur[:, k0:k0 + SG, :],
                            pet[:, :, :],
                            adj_sbs[sg][:, :, :],
                        )
                if phase >= 1:
                    for rc in range(bs_prev // 128):
                        recip = small_pool.tile([128, 1], mybir.dt.float32,
                                                tag="recip",
                                                name=f"recip_{phase}_{rc}")
                        nc.vector.reciprocal(recip[:], psO[rc][:, D:D + 1])
                        o_sb = out_pool.tile([128, D], mybir.dt.float32, tag="o",
                                             name=f"o_{phase}_{rc}")
                        nc.vector.tensor_scalar_mul(o_sb[:], psO[rc][:, 0:D],
                                                    recip[:])
                        r0 = off_prev + rc * 128
                        nc.sync.dma_start(out=o_d[r0:r0 + 128, :], in_=o_sb[:])
                ptm_prev = ptm_cur
                bs_prev = bs
                off_prev = off
    nc.finalize()
    return nc


def _get_nc():
    key = str(sorted(CFG.items()))
    if key not in _CACHE:
        _CACHE[key] = _build_nc(CFG)
    return _CACHE[key]


def _np_dt(name):
    import ml_dtypes

    return {
        "float32": np.float32,
        "bfloat16": ml_dtypes.bfloat16,
        "float16": np.float16,
        "float8e4": ml_dtypes.float8_e4m3,
        "float8e5": ml_dtypes.float8_e5m2,
    }[name]


def make_in_maps(input, adj):
    """Host-side shard/layout prep: one input map per core."""
    input = np.asarray(input, dtype=np.float32)
    adj = np.asarray(adj)
    p_np = _np_dt(CFG["p_dt"])
    am_np = _np_dt(CFG["act_mask_dt"])
    dm_np = _np_dt(CFG["dve_mask_dt"])
    recs, tot = _mask_chunks(CFG)

    in_maps = []
    for core in range(8):
        b, h = core // 2, core % 2
        xb = input[b]                                    # [N, D]
        roll = np.concatenate([np.arange(h * R, N), np.arange(0, h * R)])
        xr = xb[roll]                                    # keys rolled
        xt = np.ascontiguousarray(xr.T).astype(p_np, copy=False)
        xaug = np.concatenate([xr, np.ones((N, 1), np.float32)], axis=1)
        # device layout [p=key%128, ktile, D+1], contiguous per partition
        xaug = np.ascontiguousarray(
            xaug.reshape(NK, 128, D + 1).transpose(1, 0, 2)
        ).astype(p_np)
        # mask[r, j] = adj[b][h*R + r, roll[j]] > 0; rows r are global
        mrows = adj[b][h * R:(h + 1) * R][:, roll] > 0   # [R, N] bool
        adjm = np.empty((128, max(tot["act"], 1)), am_np)
        adja = np.empty((128, max(tot["dve"], 1)), dm_np)
        for phase, (off, bs) in enumerate(BLOCKS):
            for sg in range(NSG):
                kind, c0, ncols = recs[(phase, sg)]
                sub = mrows[off:off + bs, sg * SG * 128:(sg + 1) * SG * 128]
                # [bs, SG, 128] -> [128(key%128), SG, bs] -> flat (SG, bs)
                subt = sub.reshape(bs, SG, 128).transpose(2, 1, 0)
                flat = subt.reshape(128, ncols)
                if kind == "act":
                    adjm[:, c0:c0 + ncols] = flat.astype(am_np)
                else:
                    adja[:, c0:c0 + ncols] = np.where(
                        flat, MADD_UNMASK, MADD_MASK
                    ).astype(dm_np)
        in_maps.append({"xt": xt, "xaug": xaug, "adjm": adjm, "adja": adja})
    return in_maps


def run_device(in_maps, trace=False, trace_cores=None):
    import concourse.bass_utils as bass_utils

    if trace:
        bass_utils.upload_artifacts = lambda tmpdir: ""  # no bucket in sandbox
    nc = _get_nc()
    return bass_utils.run_bass_kernel_spmd(
        nc, in_maps, list(range(8)), trace=trace, trace_cores=trace_cores
    )


def kernel(input, adj):
    res = run_device(make_in_maps(input, adj))
    out = np.empty((B, N, D), dtype=np.float32)
    for core in range(8):
        b, h = core // 2, core % 2
        out[b, h * R:(h + 1) * R, :] = res.results[core]["o"]
    return out
